# revision 8
# baseline (speedup 1.0000x reference)
"""Tensor-parallel MultiHeadAttention (QKV + RoPE + GQA causal SDPA + dense)
for 8 Trainium2 NeuronCores — bf16 edition.

Sharding (TP as in TPMultiHeadAttention): core d owns query heads {2d, 2d+1}
and the single kv head d//2 (kv heads replicated across core pairs), plus the
matching 256 columns of the dense projection. Each core produces a full-shape
partial output; the all-reduce is a host-side sum over the 8 bf16 partials.

All matmul operands are bf16 (same PE rate as float32r at 2.4GHz, but half the
LDWEIGHTS time, half the DMA bytes, and 2x DVE rate); PSUM accumulation stays
fp32.  Per-core device pipeline:

  1. qkv^T = W_shard @ x^T  -> [f=512, s=2048], 4 PSUM banks (q0,q1,k,v),
     contraction over 16 embedding tiles; x arrives in 512KB DMAs.
  2. RoPE without PE work: qt = copy(psum) on ScalarE; the rotate-half is two
     partition-shifted multiplies on the Pool engine (tt[0:64] = qt[64:128] *
     -sin, tt[64:128] = qt[0:64] * sin); DVE combines dst = qt*cos + tt.
  3. S^T[sk, sq] = k^T.T @ q per 128-row sk tile; diagonal tiles compute only
     their causally visible query range (N in {512,384,256,128}).  P^T =
     exp(S^T) on ScalarE (scores bounded ~|5|, no max subtraction); the
     single 128-wide triangle is masked multiplicatively after exp.
  4. ctx^T[d, sq] += v_nat[sk, d].T @ P^T.  Softmax denominators: P^T tiles
     summed on DVE (two chains), column-summed with a ones-vector matmul,
     reciprocal'd on DVE, Pool-broadcast over partitions, multiplied into
     ctx^T during the PSUM->SBUF move.
  5. out[s, e] += ctx^T.T @ wd^T (2 local heads); 4 e-tiles batched per
     512KB output DMA.
"""

import numpy as np
import ml_dtypes

B, S, E = 1, 2048, 2048
H, KVH, D = 16, 4, 128
NCORES = 8
P = 128
FD = 512            # matmul moving free dim == one fp32 PSUM bank
NE = E // P         # 16 contraction tiles over the embedding dim
NG = 4              # eo-groups of 4 (one 512KB DMA each)
NSC = S // FD       # 4 sequence chunks
NST = S // P        # 16 sequence tiles
FLOC = 4 * P        # local fused qkv rows per core (2 q heads + k + v)
ROPE_BASE = 10000.0
# causally visible query sub-range start for diagonal sk tile o
DIAG_START = (0, 128, 256, 384)
BF = ml_dtypes.bfloat16

LAST_RESULT = None
_BASS_CACHE = None


def _rope_tables():
    inv = 1.0 / (ROPE_BASE ** (np.arange(0, D, 2, dtype=np.float64) / D))
    t = np.arange(S, dtype=np.float64)
    freqs = np.outer(t, inv)
    emb = np.concatenate([freqs, freqs], axis=-1)  # [S, D]
    return np.cos(emb), np.sin(emb)


def _host_constants():
    cos, sin = _rope_tables()
    cos_ds = np.ascontiguousarray(cos.T)  # [D, S]
    sin_ds = np.ascontiguousarray(sin.T)
    # sign-folded sin for the partition-shifted rotate-half:
    # tt[d] = qt[(d+64)%128] * sg[d],  sg = [-sin[:64]; +sin[64:]]
    sg = np.concatenate([-sin_ds[:64], sin_ds[64:]], axis=0)
    r_idx = np.arange(P)[:, None]
    c_idx = np.arange(P)[None, :]
    tri = (r_idx <= c_idx).astype(np.float64)
    return {
        "cosr": cos_ds.astype(BF),
        "sgsin": sg.astype(BF),
        "trim": tri.astype(BF),
        "ident": np.eye(P).astype(BF),
        "ones": np.ones((P, 1), np.float64).astype(BF),
    }


def _build_bass():
    import concourse.mybir as mybir
    import concourse.tile as tile
    from concourse import bacc

    f32 = mybir.dt.float32
    bf16 = mybir.dt.bfloat16
    Exp = mybir.ActivationFunctionType.Exp

    nc = bacc.Bacc(None, target_bir_lowering=False, name="mha_tp8_bf16")
    # x pre-tiled on host to [sc, g, p, j, f]: each [p, 4, 512] block is fully
    # contiguous per partition -> 512KB DMAs with 4KB descriptors
    xG = nc.dram_tensor("xG", [NSC, NG, P, 4, FD], bf16, kind="ExternalInput")
    wG = nc.dram_tensor("wG", [NG, P, 4, FLOC], bf16, kind="ExternalInput")
    wdG = nc.dram_tensor("wdG", [P, 2, S], bf16, kind="ExternalInput")
    cosr = nc.dram_tensor("cosr", [P, S], bf16, kind="ExternalInput")
    sgsin = nc.dram_tensor("sgsin", [P, S], bf16, kind="ExternalInput")
    trim = nc.dram_tensor("trim", [P, P], bf16, kind="ExternalInput")
    ident = nc.dram_tensor("ident", [P, P], bf16, kind="ExternalInput")
    ones = nc.dram_tensor("ones", [P, 1], bf16, kind="ExternalInput")
    # output tiled [c, st, p, eo, f]; host reassembles by plain reshape
    out = nc.dram_tensor("out", [NSC, 4, P, 4, FD], bf16, kind="ExternalOutput")

    with tile.TileContext(nc) as tc:
        with tc.tile_pool(name="const", bufs=1) as const:
            w_sb = const.tile([P, NE, FLOC], bf16, name="w_sb")
            idn = const.tile([P, P], bf16, name="idn")
            on = const.tile([P, 1], bf16, name="on")

            cq = const.tile([P, S], bf16, name="cq")
            sg = const.tile([P, S], bf16, name="sg")
            mk = const.tile([P, P], bf16, name="mk")
            wd_sb = const.tile([P, 2, S], bf16, name="wd_sb")

            qr = const.tile([P, 2, S], bf16, name="qr")
            kr = const.tile([P, S], bf16, name="kr")
            vT = const.tile([P, S], bf16, name="vT")
            vn = const.tile([P, NST, P], bf16, name="vn")

            # ---- Phase A: fused QKV projection + RoPE + v transpose ----
            with tc.tile_pool(name="xs_p", bufs=4) as xpool, \
                 tc.tile_pool(name="ps_qkv", bufs=1, space="PSUM") as pqkv, \
                 tc.tile_pool(name="ps_vt", bufs=2, space="PSUM") as pvt, \
                 tc.tile_pool(name="rtmp", bufs=3) as rtmp:
                # tables + dense weights on the scalar ring; weights and x
                # interleave on the sync ring so QKV starts within ~3us
                nc.scalar.dma_start(cq, cosr[:, :])
                nc.scalar.dma_start(sg, sgsin[:, :])
                nc.scalar.dma_start(mk, trim[:, :])
                nc.scalar.dma_start(wd_sb, wdG[:, :, :])
                for sc in range(NSC):
                    psums = [
                        pqkv.tile([P, FD], f32, tag=f"qkv{f}", name=f"ps_qkv{f}_{sc}")
                        for f in range(4)
                    ]
                    for g in range(NG):
                        if sc == 0:
                            nc.sync.dma_start(w_sb[:, 4 * g:4 * g + 4, :], wG[g])
                        xs = xpool.tile([P, 4, FD], bf16, tag="xs", name=f"xs_{sc}_{g}")
                        nc.sync.dma_start(xs, xG[sc, g])
                        if sc == 0 and g == 0:
                            nc.sync.dma_start(idn, ident[:, :])
                            nc.sync.dma_start(on, ones[:, :])
                        for j in range(4):
                            eo = 4 * g + j
                            for f in range(4):
                                nc.tensor.matmul(
                                    psums[f],
                                    lhsT=w_sb[:, eo, f * P:(f + 1) * P],
                                    rhs=xs[:, j, :],
                                    start=(eo == 0),
                                    stop=(eo == NE - 1),
                                )
                    ssl = slice(sc * FD, (sc + 1) * FD)
                    for f in range(3):
                        dst = qr[:, f, ssl] if f < 2 else kr[:, ssl]
                        qt = rtmp.tile([P, FD], bf16, tag="qt", name=f"qt_{sc}_{f}")
                        nc.scalar.copy(qt, psums[f])
                        # rotate_half as two partition-shifted copies on DVE
                        # (binary ops must share a base partition, copies
                        # not; gpsimd runs these in software at ~2us each)
                        ts = rtmp.tile([P, FD], bf16, tag="ts", name=f"ts_{sc}_{f}")
                        nc.vector.tensor_copy(ts[0:64, :], qt[64:128, :])
                        nc.vector.tensor_copy(ts[64:128, :], qt[0:64, :])
                        tt = rtmp.tile([P, FD], bf16, tag="tt", name=f"tt_{sc}_{f}")
                        nc.vector.tensor_mul(tt, ts, sg[:, ssl])
                        nc.vector.tensor_mul(dst, qt, cq[:, ssl])
                        nc.vector.tensor_add(dst, dst, tt)
                    nc.scalar.copy(vT[:, ssl], psums[3])
                    for jj in range(4):
                        j = 4 * sc + jj
                        vp = pvt.tile([P, P], bf16, tag="vt", name=f"vt_{j}")
                        nc.tensor.transpose(vp, vT[:, j * P:(j + 1) * P], idn)
                        nc.scalar.copy(vn[:, j, :], vp)  # gpsimd can't read PSUM

            # ---- Phase B: attention + dense, per 512-query chunk ----
            # Attention is software-pipelined: the two heads interleave, each
            # ctx matmul lags its score matmul by LAG pair-slots (covering the
            # ~620ns exp latency), and dense units of chunk c-1 are metered
            # into chunk c's pair loop because exp (616ns) is slower than a
            # score+ctx matmul pair (432ns) — the PE needs the extra work.
            from collections import deque

            with tc.tile_pool(name="ps_s", bufs=3, space="PSUM") as ps_s, \
                 tc.tile_pool(name="ps_ctx", bufs=2, space="PSUM") as ps_ctx, \
                 tc.tile_pool(name="ps_r", bufs=1, space="PSUM") as ps_r, \
                 tc.tile_pool(name="ps_o", bufs=2, space="PSUM") as ps_o, \
                 tc.tile_pool(name="pt_p", bufs=5) as ptp, \
                 tc.tile_pool(name="acc_p", bufs=3) as accp, \
                 tc.tile_pool(name="rb_p", bufs=3) as rbp, \
                 tc.tile_pool(name="ctx_p", bufs=3) as ctxp, \
                 tc.tile_pool(name="out_p", bufs=3) as outp:
                all_csb = {}
                LAG = 2

                def dense_units(c, tail):
                    # 16 units of (2 matmuls + 1 copy), one 512KB DMA per st;
                    # copies go to DVE while exp owns ScalarE, except in the
                    # final non-interleaved chunk where they split
                    for st in range(4):
                        ot = outp.tile([P, 4, FD], bf16, tag="ot", name=f"ot_{c}_{st}")
                        for eo in range(4):
                            op = ps_o.tile([P, FD], f32, tag="o", name=f"o_{c}_{st}_{eo}")
                            for h in range(2):
                                nc.tensor.matmul(
                                    op,
                                    lhsT=all_csb[(c, h)][:, st * P:(st + 1) * P],
                                    rhs=wd_sb[:, h, eo * FD:(eo + 1) * FD],
                                    start=(h == 0), stop=(h == 1),
                                )
                            if tail and eo % 2:
                                nc.scalar.copy(ot[:, eo, :], op)
                            else:
                                nc.vector.tensor_copy(ot[:, eo, :], op)
                            if eo == 3:
                                nc.sync.dma_start(out[c, st], ot)
                            yield

                def emit_attn(c, dq=None, dskip=6):
                    qbase = c * FD
                    nj = 4 * c + 4
                    two_chain = c >= 1
                    ctxps, accs = {}, {}
                    for h in range(2):
                        ctxps[h] = ps_ctx.tile([P, FD], f32, tag="ctx", name=f"ctx_{c}_{h}")
                        acc_a = accp.tile([P, FD], bf16, tag=f"acca{h}", name=f"acca_{c}_{h}")
                        acc_b = (
                            accp.tile([P, FD], bf16, tag=f"accb{h}", name=f"accb_{c}_{h}")
                            if two_chain else None
                        )
                        accs[h] = (acc_a, acc_b)

                    def emit_ctx(ent):
                        j, h, pt, so, n = ent
                        nc.tensor.matmul(
                            ctxps[h][:, so:],
                            lhsT=vn[:, j, :],
                            rhs=pt[:, :n],
                            start=(j == 0), stop=(j == nj - 1),
                        )

                    pairs = [(j, h) for j in range(nj) for h in range(2)]
                    np_ = len(pairs)
                    pend = deque()
                    nd = 0
                    for idx, (j, h) in enumerate(pairs):
                        o = j - 4 * c
                        so = DIAG_START[o] if o >= 0 else 0
                        n = FD - so
                        sp = ps_s.tile([P, FD], f32, tag="s", name=f"s_{c}_{h}_{j}")
                        nc.tensor.matmul(
                            sp[:, :n],
                            lhsT=kr[:, j * P:(j + 1) * P],
                            rhs=qr[:, h, qbase + so: qbase + FD],
                            start=True, stop=True,
                        )
                        pt = ptp.tile([P, FD], bf16, tag="pt", name=f"pt_{c}_{h}_{j}")
                        nc.scalar.activation(pt[:, :n], sp[:, :n], Exp)
                        if o >= 0:
                            # the partial triangle is always the first 128
                            # visible query columns of a diagonal tile
                            nc.vector.tensor_mul(pt[:, :P], pt[:, :P], mk)
                        acc_a, acc_b = accs[h]
                        acc = acc_b if (two_chain and j % 2) else acc_a
                        if j < (2 if two_chain else 1):
                            nc.vector.tensor_copy(acc, pt)
                        else:
                            nc.vector.tensor_add(acc[:, so:], acc[:, so:], pt[:, :n])
                        if dq is not None and idx >= dskip:
                            want = (idx + 1 - dskip) * 16 // (np_ - dskip)
                            while nd < want:
                                next(dq)
                                nd += 1
                        if len(pend) >= LAG + 1:
                            emit_ctx(pend.popleft())
                        pend.append((j, h, pt, so, n))
                    while pend:
                        emit_ctx(pend.popleft())
                    if dq is not None:
                        for _ in dq:
                            nd += 1
                    # softmax tails after both heads' tile loops
                    for h in range(2):
                        acc_a, acc_b = accs[h]
                        rp_ = ps_r.tile([1, FD], f32, tag="r", name=f"r_{c}_{h}")
                        if two_chain:
                            nc.tensor.matmul(rp_, lhsT=on, rhs=acc_a, start=True, stop=False)
                            nc.tensor.matmul(rp_, lhsT=on, rhs=acc_b, start=False, stop=True)
                        else:
                            nc.tensor.matmul(rp_, lhsT=on, rhs=acc_a, start=True, stop=True)
                        rec = rbp.tile([1, FD], f32, tag="rec", name=f"rec_{c}_{h}")
                        nc.vector.reciprocal_approx_fast(rec, rp_)
                        rb = rbp.tile([P, FD], f32, tag="rb", name=f"rb_{c}_{h}")
                        nc.gpsimd.partition_broadcast(rb, rec)
                        ct = ctxp.tile([P, FD], bf16, tag=f"ctx{h}", name=f"csb_{c}_{h}")
                        nc.vector.tensor_mul(ct, ctxps[h], rb)
                        all_csb[(c, h)] = ct

                emit_attn(0)
                emit_attn(1, dense_units(0, False))
                emit_attn(2, dense_units(1, False))
                emit_attn(3, dense_units(2, False))
                for _ in dense_units(3, True):
                    pass
    nc.compile()
    return nc


def make_in_maps(x, w_qkv, w_dense):
    x = np.asarray(x, np.float32).reshape(S, E)
    w_qkv = np.asarray(w_qkv, np.float32)
    w_dense = np.asarray(w_dense, np.float32)
    # x^T tiled to [sc, g, p, j, f] so each 512KB DMA block is contiguous
    xT = np.ascontiguousarray(x.T)
    xG = np.ascontiguousarray(
        xT.reshape(NG, 4, P, NSC, FD).transpose(3, 0, 2, 1, 4)
    ).astype(BF)
    consts = _host_constants()
    in_maps = []
    scale = np.float64(1.0 / np.sqrt(D))
    for d in range(NCORES):
        g = d // 2
        wq = w_qkv[2 * d * P:(2 * d + 2) * P] * scale
        wk = w_qkv[H * D + g * P: H * D + (g + 1) * P]
        wv = w_qkv[H * D + KVH * D + g * P: H * D + KVH * D + (g + 1) * P]
        wqkvT_d = np.ascontiguousarray(np.concatenate([wq, wk, wv], 0).T)
        wG_d = np.ascontiguousarray(
            wqkvT_d.reshape(NG, 4, P, FLOC).transpose(0, 2, 1, 3)
        ).astype(BF)
        wdT_d = w_dense[:, 2 * d * P:(2 * d + 2) * P].T  # [2P, S]
        wdG_d = np.ascontiguousarray(
            wdT_d.reshape(2, P, S).transpose(1, 0, 2)
        ).astype(BF)
        m = {"xG": xG, "wG": wG_d, "wdG": wdG_d}
        m.update(consts)
        in_maps.append(m)
    return in_maps


def kernel(x, w_qkv, w_dense):
    global LAST_RESULT, _BASS_CACHE
    from concourse.bass_utils import run_bass_kernel_spmd

    in_maps = make_in_maps(x, w_qkv, w_dense)
    if _BASS_CACHE is None:
        _BASS_CACHE = _build_bass()
    res = run_bass_kernel_spmd(_BASS_CACHE, in_maps, core_ids=list(range(NCORES)))
    LAST_RESULT = res
    # sum partials over cores; [c, st, p, eo, f] flattens straight to [s, e]
    acc = np.zeros((NSC, 4, P, 4, FD), np.float32)
    for r in res.results:
        acc += r["out"].astype(np.float32)
    return np.ascontiguousarray(acc.reshape(S, E)).reshape(B, S, E)
